# revision 17
# baseline (speedup 1.0000x reference)
"""Trainium2 Bass kernel for nn_CP2_17669495456475 (dynamic-kernel deconv).

Math: out[b,c,y,x] = sum_l cos[b,l,i,j] * W[b,l,c,ky,kx],  y=8i+ky, x=8j+kx,
with W = unfold(pad(b)) * (1 - unfold(pad(mask))), K=16, S=8, crop 4.

Decomposition (per core): since K = 2*S, split ky = ry + 8*sy, kx = rx + 8*sx.
With u = i+sy, v = j+sx the whole op is ONE matmul with contraction over
(a,sy,sx,p) -> (l,sy,sx) of size 4096:

  outT[(c,ry,rx), (u,v)] = sum_{l,sy,sx} bm_block[(li+sy, lj+sx), (c,ry,rx)]
                                          * Xp[l, 1+u-sy, 1+v-sx]

where bm = pad(b)*(1-pad(mask)) laid out in 8x8 blocks (the unfold becomes
duplication-free shifted block views) and the deconv overlap-add is absorbed
into PSUM accumulation.  The mask multiply is fused on-device (DVE) on the
gathered W chunk tiles.

Sharding: 8 cores = 4 batches x 2 channel-halves (16 ch each). Full inputs in,
full output out; host does layout glue (replicate pad, block reshape, zero pad,
final crop/assembly) only.
"""

import os
import numpy as np

import concourse.bass as bass
import concourse.mybir as mybir
import concourse.tile as tile
from concourse.bass_utils import run_bass_kernel_spmd

PD = 4
C = 16              # channels per core
N_CORES = 8
CHUNKS = [(a, sy, sx) for a in range(8) for sy in (0, 1) for sx in (0, 1)]

# matmul input dtype: "f32" (exact, 4 cyc/row), "f32r" (relaxed, 1 cyc/row,
# slow 4-byte weight loads), or "bf16" (1 cyc/row + fast weight load)
MM_DTYPE = os.environ.get("BASSK_MM_DTYPE", "bf16")
# fp32r requires an even innermost moving count -> keep the dead v=33 col;
# bf16/f32 have no such restriction.
NV = 34 if MM_DTYPE == "f32r" else 33
NT = 11 * NV          # N per matmul: 11 u-rows x NV v-cols


def _split_multi_sync(nc):
    """The walrus in this env allows only ONE sync-wait per instruction.
    Hoist extra waits onto same-engine InstNoOp carriers placed just before
    the owning instruction (sequential waits on one engine == AND)."""
    ctr = 0
    for f in nc.m.functions:
        for bb in f.blocks:
            insts = list(bb.instructions)
            out = []
            changed = False
            for inst in insts:
                si = inst.sync_info
                waits = list(si.on_wait) if si and si.on_wait else []
                if len(waits) > 1:
                    for w in waits[:-1]:
                        nop = mybir.InstNoOp(name=f"waitnop-{ctr}", ins=[], outs=[])
                        ctr += 1
                        nop.engine = inst.engine
                        nop.sync_info = mybir.SyncInfo(on_wait=[w], on_update=[])
                        out.append(nop)
                    si.on_wait = [waits[-1]]
                    changed = True
                out.append(inst)
            if changed:
                bb.instructions = out
    return ctr


def _build_nc():
    f32 = mybir.dt.float32
    # float32r has identical bits/np-dtype to float32; the PE runs its
    # matmuls at 1 cyc/row (vs 4 for exact f32). The BIR verifier requires
    # every producer of an f32r matmul operand to *output* f32r, so the
    # W/X dram params and sbuf tiles are typed f32r end-to-end.
    mmdt = {"f32r": mybir.dt.float32r, "bf16": mybir.dt.bfloat16}.get(MM_DTYPE, f32)
    nc = bass.Bass(enable_partition_id=False)
    # W chunks pre-gathered host-side, partition-major: [p, ci, (c,ry,rx)].
    # 16KB contiguous per partition per 4-chunk quad -> large DMA packets
    # (4KB runs cost ~400ns/packet overhead and cap DMA at ~160 GB/s).
    w4 = nc.declare_dram_parameter("w4", [128, 32, C * 64], mmdt, isOutput=False)
    mT = nc.declare_dram_parameter("mT", [128, 32, 64], f32, isOutput=False)
    # X is y-major [p, yy, a, xx] and loads in 3 phase-aligned y-slabs:
    # phase n only reads rows [11n, 11n+13), so the first matmuls need just
    # slab 0 (rows 0..13) instead of the whole 5MB tensor.
    xp = nc.declare_dram_parameter("xp", [128, 34, 8, 36], mmdt, isOutput=False)
    # a-major copy of slab-0 rows so the startup slivers are contiguous
    xp0 = nc.declare_dram_parameter("xp0", [128, 8, 13, 36], mmdt, isOutput=False)
    # out: phase-major [n, p, m, NT] so each phase writes one 12KB/partition DMA
    outT = nc.declare_dram_parameter("outT", [3, 128, 8, NT], f32, isOutput=True)

    with tile.TileContext(nc) as tc:
        with (
            tc.tile_pool(name="xpp", bufs=1) as xpp,
            tc.tile_pool(name="wp", bufs=8) as wp,
            tc.tile_pool(name="mp", bufs=1) as mp,
            tc.tile_pool(name="onp", bufs=4) as onp,
            tc.tile_pool(name="op", bufs=2) as op,
            tc.tile_pool(name="pp", bufs=8, space="PSUM") as pp,
        ):
            # Startup-critical DMAs lead each queue's FIFO: the first matmul
            # needs only s0a (a=0; chunks 0-3 all read a=0) + mta0 + chunk 0
            # of W on the sync queue.
            # one tile per y-slab (separate tiles: dependency tracking is
            # whole-tile, and deferred slabs must not deadlock phase-0 reads).
            # Slabs overlap by 2 rows so each phase reads within one slab.
            SLABS = ((0, 13), (11, 24), (22, 34))
            s0a = xpp.tile([128, 1, 13, 36], mmdt)
            nc.scalar.dma_start(s0a[:], xp0[:, 0:1])
            mta0 = mp.tile([128, 4, 64], f32)
            nc.scalar.dma_start(mta0[:], mT[:, 0:4])
            s0b = xpp.tile([128, 7, 13, 36], mmdt)
            nc.scalar.dma_start(s0b[:], xp0[:, 1:8])
            mta1 = mp.tile([128, 28, 64], f32)
            nc.scalar.dma_start(mta1[:], mT[:, 4:32])

            def mask_ap(ci):
                return mta0[:, ci, :] if ci < 4 else mta1[:, ci - 4, :]
            slab_dmas = [None]
            slabs = [None]
            for si, (y0, y1) in list(enumerate(SLABS))[1:]:
                st = xpp.tile([128, y1 - y0, 8, 36], mmdt, name=f"slab_{si}")
                slab_dmas.append(nc.scalar.dma_start(st[:], xp[:, y0:y1]))
                slabs.append(st)

            def rhs_ap(n, a, sy, sx):
                y0 = 11 * n + 1 - sy - SLABS[n][0]
                x0 = 1 - sx
                if n == 0:
                    t, aa = (s0a, a) if a < 1 else (s0b, a - 1)
                    return t[:, aa, y0:y0 + 11, x0:x0 + NV]
                return slabs[n][:, y0:y0 + 11, a, x0:x0 + NV]

            # W streams in 8 quads of 4 chunks; mask-mul fused per chunk.
            # Chunk 0 gets its OWN tile: dependency tracking is whole-tile,
            # so as a quad slice its first matmul would wait on all 4 muls.
            wc0 = wp.tile([128, 1, C * 64], mmdt, name="wc0")
            nc.sync.dma_start(wc0[:], w4[:, 0:1, :])
            quads = []
            quad_dmas = []
            for g in range(8):
                wq = wp.tile([128, 4, C * 64], mmdt, tag="w", name=f"wq_{g}")
                if g == 0:
                    dma = nc.sync.dma_start(wq[:, 1:4, :], w4[:, 1:4, :])
                else:
                    dma = nc.sync.dma_start(wq[:], w4[:, 4 * g:4 * g + 4, :])
                quad_dmas.append(dma)
                for j in range(4):
                    ci = 4 * g + j
                    om = onp.tile([128, 64], f32, tag="om")
                    nc.vector.tensor_scalar(
                        out=om[:], in0=mask_ap(ci), scalar1=-1.0, scalar2=1.0,
                        op0=mybir.AluOpType.mult, op1=mybir.AluOpType.add,
                    )
                    wt_dst = wc0 if ci == 0 else wq
                    jj = 0 if ci == 0 else j
                    wv = wt_dst[:, jj, :].rearrange("p (c f) -> p c f", c=C)
                    in0 = wv.bitcast(f32) if MM_DTYPE == "f32r" else wv
                    nc.vector.tensor_tensor(
                        out=wv, in0=in0,
                        in1=om[:, None, :].broadcast_to([128, C, 64]),
                        op=mybir.AluOpType.mult,
                    )
                quads.append(wq)

            def lhsT(ci, m):
                if ci == 0:
                    return wc0[:, 0, 128 * m:128 * (m + 1)]
                return quads[ci // 4][:, ci % 4, 128 * m:128 * (m + 1)]

            # Phase 0 (n=0) is chunk-outer with 8 live psum groups so the PE
            # consumes W quads as they stream (no all-32-chunks stall).
            # Phases 1-2 run m-outer (all data resident) so each group's
            # psum copy + output DMA overlaps the next group's matmuls.
            for n in range(3):
                osb = op.tile([128, 8, NT], f32, tag="o", name=f"osb_{n}")
                if n == 0:
                    pss = [pp.tile([128, NT], f32, tag="ps", name=f"ps_{n}_{i}")
                           for i in range(8)]
                    mm0 = {}
                    for ci, (a, sy, sx) in enumerate(CHUNKS):
                        rhs = rhs_ap(n, a, sy, sx)
                        for m in range(8):
                            mm0[ci, m] = nc.tensor.matmul(
                                pss[m][:], lhsT(ci, m), rhs,
                                start=(ci == 0), stop=(ci == 31),
                            )
                    # Defer the late W quads / X slabs behind PE progress so
                    # the startup-critical DMAs (mta, quad0, slab0) get the
                    # full DMA bandwidth during the ramp.
                    from concourse.tile_rust import add_dep_helper
                    add_dep_helper(quad_dmas[2].ins, mm0[0, 0].ins,
                                   sync=True, reason="stream quads behind PE")
                    for g in range(3, 8):
                        add_dep_helper(quad_dmas[g].ins, mm0[4 * (g - 3) + 3, 7].ins,
                                       sync=True, reason="stream quads behind PE")
                    add_dep_helper(slab_dmas[1].ins, mm0[3, 7].ins,
                                   sync=True, reason="slab1 after early phase0")
                    add_dep_helper(slab_dmas[2].ins, mm0[11, 7].ins,
                                   sync=True, reason="slab2 after mid phase0")
                    for m in range(8):
                        nc.vector.tensor_copy(osb[:, m, :], pss[m][:])
                else:
                    for m in range(8):
                        ps = pp.tile([128, NT], f32, tag="ps", name=f"ps_{n}_{m}")
                        for ci, (a, sy, sx) in enumerate(CHUNKS):
                            rhs = rhs_ap(n, a, sy, sx)
                            nc.tensor.matmul(
                                ps[:], lhsT(ci, m), rhs,
                                start=(ci == 0), stop=(ci == 31),
                            )
                        nc.vector.tensor_copy(osb[:, m, :], ps[:])
                # split the phase writeback so earlier pieces overlap the
                # remaining matmuls (finest on the last phase to shrink tail)
                if n < 2:
                    nc.scalar.dma_start(outT[n, :, 0:4], osb[:, 0:4])
                    nc.scalar.dma_start(outT[n, :, 4:8], osb[:, 4:8])
                else:
                    for mm2 in range(0, 8, 2):
                        nc.scalar.dma_start(outT[n, :, mm2:mm2 + 2],
                                            osb[:, mm2:mm2 + 2])

    _split_multi_sync(nc)
    return nc


def _host_prep(b_ch, mask_b, cos_b):
    """b_ch (16,256,256) f32, mask_b (256,256) f32, cos_b (1024,32,32) f32
    -> dict of device inputs (layout/gather glue only)."""
    bpad = np.pad(b_ch, ((0, 0), (PD, PD), (PD, PD)), mode="edge")
    mpad = np.pad(mask_b, ((PD, PD), (PD, PD)), mode="edge")
    # block layout [bi*33+bj, (c,ry,rx)]
    bT = bpad.reshape(C, 33, 8, 33, 8).transpose(1, 3, 0, 2, 4).reshape(33 * 33, C * 64)
    mTb = mpad.reshape(33, 8, 33, 8).transpose(0, 2, 1, 3).reshape(33 * 33, 64)
    # unfold-as-shifted-block-views: chunk (a,sy,sx), partition p=32*pi+pj
    # reads block row (4a+pi+sy)*33 + (pj+sx).  Pre-gather partition-major.
    pi, pj = np.arange(4)[:, None], np.arange(32)[None, :]
    rows = np.stack([((4 * a + pi + sy) * 33 + (pj + sx)).reshape(128)
                     for (a, sy, sx) in CHUNKS], axis=1)        # [128, 32]
    w4 = np.ascontiguousarray(bT[rows])                          # [128,32,1024]
    mT = np.ascontiguousarray(mTb[rows])                         # [128,32,64]
    xp = np.zeros((1024, 34, 36), np.float32)
    xp[:, 1:33, 1:33] = cos_b
    # [l=128a+p, yy, xx] -> [p, yy, a, xx]; plus an a-major slab-0 copy
    xpb = xp.reshape(8, 128, 34, 36)
    xp0 = np.ascontiguousarray(xpb[:, :, 0:13, :].transpose(1, 0, 2, 3))
    xp = np.ascontiguousarray(xpb.transpose(1, 2, 0, 3))
    if MM_DTYPE == "bf16":
        import ml_dtypes
        w4 = w4.astype(ml_dtypes.bfloat16)
        xp = xp.astype(ml_dtypes.bfloat16)
        xp0 = xp0.astype(ml_dtypes.bfloat16)
    return {"w4": w4, "mT": mT, "xp": xp, "xp0": xp0}


def _unshard(outT):
    # outT [3, 128, 8, 11*NV] -> [(c,ry,rx)=128m+p, u=11n+u', v] -> (16,256,256)
    t = outT.reshape(3, 128, 8, 11, NV).transpose(2, 1, 0, 3, 4).reshape(1024, 33, NV)
    t = t[:, :, :33].reshape(C, 8, 8, 33, 33).transpose(0, 3, 1, 4, 2)
    return t.reshape(C, 264, 264)[:, 4:260, 4:260]


_RUN_KW = {}   # test harness may inject e.g. trace=True
_LAST_RESULTS = [None]
_NC_CACHE = {}


def _get_nc():
    nc = _NC_CACHE.get(MM_DTYPE)
    if nc is None:
        nc = _NC_CACHE[MM_DTYPE] = _build_nc()
    return nc


def kernel(cos_similar, b, mask):
    cos_similar = np.ascontiguousarray(np.asarray(cos_similar, dtype=np.float32))
    b = np.ascontiguousarray(np.asarray(b, dtype=np.float32))
    mask = np.ascontiguousarray(np.asarray(mask, dtype=np.float32))

    in_maps = []
    for core in range(N_CORES):
        batch, half = core // 2, core % 2
        ch0 = C * half
        in_maps.append(_host_prep(
            b[batch, ch0:ch0 + C], mask[batch, 0], cos_similar[batch]))

    nc = _get_nc()
    res = run_bass_kernel_spmd(nc, in_maps, list(range(N_CORES)), **_RUN_KW)
    _LAST_RESULTS[0] = res

    out = np.empty((4, 32, 256, 256), np.float32)
    for core in range(N_CORES):
        batch, half = core // 2, core % 2
        ch0 = C * half
        out[batch, ch0:ch0 + C] = _unshard(res.results[core]["outT"])
    return out


# revision 18
# speedup vs baseline: 1.0087x; 1.0087x over previous
"""Trainium2 Bass kernel for nn_CP2_17669495456475 (dynamic-kernel deconv).

Math: out[b,c,y,x] = sum_l cos[b,l,i,j] * W[b,l,c,ky,kx],  y=8i+ky, x=8j+kx,
with W = unfold(pad(b)) * (1 - unfold(pad(mask))), K=16, S=8, crop 4.

Decomposition (per core): since K = 2*S, split ky = ry + 8*sy, kx = rx + 8*sx.
With u = i+sy, v = j+sx the whole op is ONE matmul with contraction over
(a,sy,sx,p) -> (l,sy,sx) of size 4096:

  outT[(c,ry,rx), (u,v)] = sum_{l,sy,sx} bm_block[(li+sy, lj+sx), (c,ry,rx)]
                                          * Xp[l, 1+u-sy, 1+v-sx]

where bm = pad(b)*(1-pad(mask)) laid out in 8x8 blocks (the unfold becomes
duplication-free shifted block views) and the deconv overlap-add is absorbed
into PSUM accumulation.  The mask multiply is fused on-device (DVE) on the
gathered W chunk tiles.

Sharding: 8 cores = 4 batches x 2 channel-halves (16 ch each). Full inputs in,
full output out; host does layout glue (replicate pad, block reshape, zero pad,
final crop/assembly) only.
"""

import os
import numpy as np

import concourse.bass as bass
import concourse.mybir as mybir
import concourse.tile as tile
from concourse.bass_utils import run_bass_kernel_spmd

PD = 4
C = 16              # channels per core
N_CORES = 8
CHUNKS = [(a, sy, sx) for a in range(8) for sy in (0, 1) for sx in (0, 1)]

# matmul input dtype: "f32" (exact, 4 cyc/row), "f32r" (relaxed, 1 cyc/row,
# slow 4-byte weight loads), or "bf16" (1 cyc/row + fast weight load)
MM_DTYPE = os.environ.get("BASSK_MM_DTYPE", "bf16")
# fp32r requires an even innermost moving count -> keep the dead v=33 col;
# bf16/f32 have no such restriction.
NV = 34 if MM_DTYPE == "f32r" else 33
NT = 11 * NV          # N per matmul: 11 u-rows x NV v-cols


def _split_multi_sync(nc):
    """The walrus in this env allows only ONE sync-wait per instruction.
    Hoist extra waits onto same-engine InstNoOp carriers placed just before
    the owning instruction (sequential waits on one engine == AND)."""
    ctr = 0
    for f in nc.m.functions:
        for bb in f.blocks:
            insts = list(bb.instructions)
            out = []
            changed = False
            for inst in insts:
                si = inst.sync_info
                waits = list(si.on_wait) if si and si.on_wait else []
                if len(waits) > 1:
                    for w in waits[:-1]:
                        nop = mybir.InstNoOp(name=f"waitnop-{ctr}", ins=[], outs=[])
                        ctr += 1
                        nop.engine = inst.engine
                        nop.sync_info = mybir.SyncInfo(on_wait=[w], on_update=[])
                        out.append(nop)
                    si.on_wait = [waits[-1]]
                    changed = True
                out.append(inst)
            if changed:
                bb.instructions = out
    return ctr


def _build_nc():
    f32 = mybir.dt.float32
    # float32r has identical bits/np-dtype to float32; the PE runs its
    # matmuls at 1 cyc/row (vs 4 for exact f32). The BIR verifier requires
    # every producer of an f32r matmul operand to *output* f32r, so the
    # W/X dram params and sbuf tiles are typed f32r end-to-end.
    mmdt = {"f32r": mybir.dt.float32r, "bf16": mybir.dt.bfloat16}.get(MM_DTYPE, f32)
    nc = bass.Bass(enable_partition_id=False)
    # W chunks pre-gathered host-side, partition-major: [p, ci, (c,ry,rx)].
    # 16KB contiguous per partition per 4-chunk quad -> large DMA packets
    # (4KB runs cost ~400ns/packet overhead and cap DMA at ~160 GB/s).
    w4 = nc.declare_dram_parameter("w4", [128, 32, C * 64], mmdt, isOutput=False)
    mT = nc.declare_dram_parameter("mT", [128, 32, 64], f32, isOutput=False)
    # X is y-major [p, yy, a, xx] and loads in 3 phase-aligned y-slabs:
    # phase n only reads rows [11n, 11n+13), so the first matmuls need just
    # slab 0 (rows 0..13) instead of the whole 5MB tensor.
    xp = nc.declare_dram_parameter("xp", [128, 34, 8, 36], mmdt, isOutput=False)
    # a-major copy of slab-0 rows so the startup slivers are contiguous
    xp0 = nc.declare_dram_parameter("xp0", [128, 8, 13, 36], mmdt, isOutput=False)
    # out: phase-major [n, p, m, NT] so each phase writes one 12KB/partition DMA
    outT = nc.declare_dram_parameter("outT", [3, 128, 8, NT], f32, isOutput=True)

    with tile.TileContext(nc) as tc:
        with (
            tc.tile_pool(name="xpp", bufs=1) as xpp,
            tc.tile_pool(name="wp", bufs=8) as wp,
            tc.tile_pool(name="mp", bufs=1) as mp,
            tc.tile_pool(name="onp", bufs=4) as onp,
            tc.tile_pool(name="op", bufs=2) as op,
            tc.tile_pool(name="pp", bufs=8, space="PSUM") as pp,
        ):
            # PE pre-warm: ~20 dummy matmuls on a memset tile fill the
            # otherwise-idle window while input DMAs land, tripping the HAM
            # clock gate to 8/8 before the real stream starts.  The psum
            # tile shares the "ps" slots and is released before phase 0
            # needs all 8.  Tiny leading DMAs absorb each ring's first-
            # transfer warmup so the startup-critical loads behind them
            # complete sooner.
            wrm = mp.tile([128, 512], mmdt, name="warm")
            nc.gpsimd.memset(wrm[:], 0.0)
            ps_warm = pp.tile([128, 512], f32, tag="ps", name="ps_warm")
            for _ in range(20):
                nc.tensor.matmul(ps_warm[:], wrm[:, 0:128], wrm[:],
                                 start=True, stop=True)
            warm_dma = mp.tile([128, 8], f32, name="warm_dma")
            nc.sync.dma_start(warm_dma[:, 0:4], mT[:, 0, 0:4])
            nc.scalar.dma_start(warm_dma[:, 4:8], mT[:, 0, 4:8])
            # Startup-critical DMAs lead each queue's FIFO: the first matmul
            # needs only s0a (a=0; chunks 0-3 all read a=0) + mta0 + chunk 0
            # of W on the sync queue.
            # one tile per y-slab (separate tiles: dependency tracking is
            # whole-tile, and deferred slabs must not deadlock phase-0 reads).
            # Slabs overlap by 2 rows so each phase reads within one slab.
            SLABS = ((0, 13), (11, 24), (22, 34))
            s0a = xpp.tile([128, 1, 13, 36], mmdt)
            nc.scalar.dma_start(s0a[:], xp0[:, 0:1])
            mta0 = mp.tile([128, 4, 64], f32)
            nc.scalar.dma_start(mta0[:], mT[:, 0:4])
            s0b = xpp.tile([128, 7, 13, 36], mmdt)
            nc.scalar.dma_start(s0b[:], xp0[:, 1:8])
            mta1 = mp.tile([128, 28, 64], f32)
            nc.scalar.dma_start(mta1[:], mT[:, 4:32])

            def mask_ap(ci):
                return mta0[:, ci, :] if ci < 4 else mta1[:, ci - 4, :]
            slab_dmas = [None]
            slabs = [None]
            for si, (y0, y1) in list(enumerate(SLABS))[1:]:
                st = xpp.tile([128, y1 - y0, 8, 36], mmdt, name=f"slab_{si}")
                slab_dmas.append(nc.scalar.dma_start(st[:], xp[:, y0:y1]))
                slabs.append(st)

            def rhs_ap(n, a, sy, sx):
                y0 = 11 * n + 1 - sy - SLABS[n][0]
                x0 = 1 - sx
                if n == 0:
                    t, aa = (s0a, a) if a < 1 else (s0b, a - 1)
                    return t[:, aa, y0:y0 + 11, x0:x0 + NV]
                return slabs[n][:, y0:y0 + 11, a, x0:x0 + NV]

            # W streams in 8 quads of 4 chunks; mask-mul fused per chunk.
            # Chunk 0 gets its OWN tile: dependency tracking is whole-tile,
            # so as a quad slice its first matmul would wait on all 4 muls.
            wc0 = wp.tile([128, 1, C * 64], mmdt, name="wc0")
            nc.sync.dma_start(wc0[:], w4[:, 0:1, :])
            quads = []
            quad_dmas = []
            for g in range(8):
                wq = wp.tile([128, 4, C * 64], mmdt, tag="w", name=f"wq_{g}")
                if g == 0:
                    dma = nc.sync.dma_start(wq[:, 1:4, :], w4[:, 1:4, :])
                else:
                    dma = nc.sync.dma_start(wq[:], w4[:, 4 * g:4 * g + 4, :])
                quad_dmas.append(dma)
                for j in range(4):
                    ci = 4 * g + j
                    om = onp.tile([128, 64], f32, tag="om")
                    nc.vector.tensor_scalar(
                        out=om[:], in0=mask_ap(ci), scalar1=-1.0, scalar2=1.0,
                        op0=mybir.AluOpType.mult, op1=mybir.AluOpType.add,
                    )
                    wt_dst = wc0 if ci == 0 else wq
                    jj = 0 if ci == 0 else j
                    wv = wt_dst[:, jj, :].rearrange("p (c f) -> p c f", c=C)
                    in0 = wv.bitcast(f32) if MM_DTYPE == "f32r" else wv
                    nc.vector.tensor_tensor(
                        out=wv, in0=in0,
                        in1=om[:, None, :].broadcast_to([128, C, 64]),
                        op=mybir.AluOpType.mult,
                    )
                quads.append(wq)

            def lhsT(ci, m):
                if ci == 0:
                    return wc0[:, 0, 128 * m:128 * (m + 1)]
                return quads[ci // 4][:, ci % 4, 128 * m:128 * (m + 1)]

            # Phase 0 (n=0) is chunk-outer with 8 live psum groups so the PE
            # consumes W quads as they stream (no all-32-chunks stall).
            # Phases 1-2 run m-outer (all data resident) so each group's
            # psum copy + output DMA overlaps the next group's matmuls.
            for n in range(3):
                osb = op.tile([128, 8, NT], f32, tag="o", name=f"osb_{n}")
                if n == 0:
                    pss = [pp.tile([128, NT], f32, tag="ps", name=f"ps_{n}_{i}")
                           for i in range(8)]
                    mm0 = {}
                    for ci, (a, sy, sx) in enumerate(CHUNKS):
                        rhs = rhs_ap(n, a, sy, sx)
                        for m in range(8):
                            mm0[ci, m] = nc.tensor.matmul(
                                pss[m][:], lhsT(ci, m), rhs,
                                start=(ci == 0), stop=(ci == 31),
                            )
                    # Defer the late W quads / X slabs behind PE progress so
                    # the startup-critical DMAs (mta, quad0, slab0) get the
                    # full DMA bandwidth during the ramp.
                    from concourse.tile_rust import add_dep_helper
                    add_dep_helper(quad_dmas[2].ins, mm0[0, 0].ins,
                                   sync=True, reason="stream quads behind PE")
                    for g in range(3, 8):
                        add_dep_helper(quad_dmas[g].ins, mm0[4 * (g - 3) + 3, 7].ins,
                                       sync=True, reason="stream quads behind PE")
                    add_dep_helper(slab_dmas[1].ins, mm0[3, 7].ins,
                                   sync=True, reason="slab1 after early phase0")
                    add_dep_helper(slab_dmas[2].ins, mm0[11, 7].ins,
                                   sync=True, reason="slab2 after mid phase0")
                    for m in range(8):
                        nc.vector.tensor_copy(osb[:, m, :], pss[m][:])
                else:
                    for m in range(8):
                        ps = pp.tile([128, NT], f32, tag="ps", name=f"ps_{n}_{m}")
                        for ci, (a, sy, sx) in enumerate(CHUNKS):
                            rhs = rhs_ap(n, a, sy, sx)
                            nc.tensor.matmul(
                                ps[:], lhsT(ci, m), rhs,
                                start=(ci == 0), stop=(ci == 31),
                            )
                        nc.vector.tensor_copy(osb[:, m, :], ps[:])
                # split the phase writeback so earlier pieces overlap the
                # remaining matmuls (finest on the last phase to shrink tail)
                if n < 2:
                    nc.scalar.dma_start(outT[n, :, 0:4], osb[:, 0:4])
                    nc.scalar.dma_start(outT[n, :, 4:8], osb[:, 4:8])
                else:
                    for mm2 in range(0, 8, 2):
                        nc.scalar.dma_start(outT[n, :, mm2:mm2 + 2],
                                            osb[:, mm2:mm2 + 2])

    _split_multi_sync(nc)
    return nc


def _host_prep(b_ch, mask_b, cos_b):
    """b_ch (16,256,256) f32, mask_b (256,256) f32, cos_b (1024,32,32) f32
    -> dict of device inputs (layout/gather glue only)."""
    bpad = np.pad(b_ch, ((0, 0), (PD, PD), (PD, PD)), mode="edge")
    mpad = np.pad(mask_b, ((PD, PD), (PD, PD)), mode="edge")
    # block layout [bi*33+bj, (c,ry,rx)]
    bT = bpad.reshape(C, 33, 8, 33, 8).transpose(1, 3, 0, 2, 4).reshape(33 * 33, C * 64)
    mTb = mpad.reshape(33, 8, 33, 8).transpose(0, 2, 1, 3).reshape(33 * 33, 64)
    # unfold-as-shifted-block-views: chunk (a,sy,sx), partition p=32*pi+pj
    # reads block row (4a+pi+sy)*33 + (pj+sx).  Pre-gather partition-major.
    pi, pj = np.arange(4)[:, None], np.arange(32)[None, :]
    rows = np.stack([((4 * a + pi + sy) * 33 + (pj + sx)).reshape(128)
                     for (a, sy, sx) in CHUNKS], axis=1)        # [128, 32]
    w4 = np.ascontiguousarray(bT[rows])                          # [128,32,1024]
    mT = np.ascontiguousarray(mTb[rows])                         # [128,32,64]
    xp = np.zeros((1024, 34, 36), np.float32)
    xp[:, 1:33, 1:33] = cos_b
    # [l=128a+p, yy, xx] -> [p, yy, a, xx]; plus an a-major slab-0 copy
    xpb = xp.reshape(8, 128, 34, 36)
    xp0 = np.ascontiguousarray(xpb[:, :, 0:13, :].transpose(1, 0, 2, 3))
    xp = np.ascontiguousarray(xpb.transpose(1, 2, 0, 3))
    if MM_DTYPE == "bf16":
        import ml_dtypes
        w4 = w4.astype(ml_dtypes.bfloat16)
        xp = xp.astype(ml_dtypes.bfloat16)
        xp0 = xp0.astype(ml_dtypes.bfloat16)
    return {"w4": w4, "mT": mT, "xp": xp, "xp0": xp0}


def _unshard(outT):
    # outT [3, 128, 8, 11*NV] -> [(c,ry,rx)=128m+p, u=11n+u', v] -> (16,256,256)
    t = outT.reshape(3, 128, 8, 11, NV).transpose(2, 1, 0, 3, 4).reshape(1024, 33, NV)
    t = t[:, :, :33].reshape(C, 8, 8, 33, 33).transpose(0, 3, 1, 4, 2)
    return t.reshape(C, 264, 264)[:, 4:260, 4:260]


_RUN_KW = {}   # test harness may inject e.g. trace=True
_LAST_RESULTS = [None]
_NC_CACHE = {}


def _get_nc():
    nc = _NC_CACHE.get(MM_DTYPE)
    if nc is None:
        nc = _NC_CACHE[MM_DTYPE] = _build_nc()
    return nc


def kernel(cos_similar, b, mask):
    cos_similar = np.ascontiguousarray(np.asarray(cos_similar, dtype=np.float32))
    b = np.ascontiguousarray(np.asarray(b, dtype=np.float32))
    mask = np.ascontiguousarray(np.asarray(mask, dtype=np.float32))

    in_maps = []
    for core in range(N_CORES):
        batch, half = core // 2, core % 2
        ch0 = C * half
        in_maps.append(_host_prep(
            b[batch, ch0:ch0 + C], mask[batch, 0], cos_similar[batch]))

    nc = _get_nc()
    res = run_bass_kernel_spmd(nc, in_maps, list(range(N_CORES)), **_RUN_KW)
    _LAST_RESULTS[0] = res

    out = np.empty((4, 32, 256, 256), np.float32)
    for core in range(N_CORES):
        batch, half = core // 2, core % 2
        ch0 = C * half
        out[batch, ch0:ch0 + C] = _unshard(res.results[core]["outT"])
    return out


# revision 19
# speedup vs baseline: 1.0157x; 1.0070x over previous
"""Trainium2 Bass kernel for nn_CP2_17669495456475 (dynamic-kernel deconv).

Math: out[b,c,y,x] = sum_l cos[b,l,i,j] * W[b,l,c,ky,kx],  y=8i+ky, x=8j+kx,
with W = unfold(pad(b)) * (1 - unfold(pad(mask))), K=16, S=8, crop 4.

Decomposition (per core): since K = 2*S, split ky = ry + 8*sy, kx = rx + 8*sx.
With u = i+sy, v = j+sx the whole op is ONE matmul with contraction over
(a,sy,sx,p) -> (l,sy,sx) of size 4096:

  outT[(c,ry,rx), (u,v)] = sum_{l,sy,sx} bm_block[(li+sy, lj+sx), (c,ry,rx)]
                                          * Xp[l, 1+u-sy, 1+v-sx]

where bm = pad(b)*(1-pad(mask)) laid out in 8x8 blocks (the unfold becomes
duplication-free shifted block views) and the deconv overlap-add is absorbed
into PSUM accumulation.  The mask multiply is fused on-device (DVE) on the
gathered W chunk tiles.

Sharding: 8 cores = 4 batches x 2 channel-halves (16 ch each). Full inputs in,
full output out; host does layout glue (replicate pad, block reshape, zero pad,
final crop/assembly) only.
"""

import os
import numpy as np

import concourse.bass as bass
import concourse.mybir as mybir
import concourse.tile as tile
from concourse.bass_utils import run_bass_kernel_spmd

PD = 4
C = 16              # channels per core
N_CORES = 8
CHUNKS = [(a, sy, sx) for a in range(8) for sy in (0, 1) for sx in (0, 1)]

# matmul input dtype: "f32" (exact, 4 cyc/row), "f32r" (relaxed, 1 cyc/row,
# slow 4-byte weight loads), or "bf16" (1 cyc/row + fast weight load)
MM_DTYPE = os.environ.get("BASSK_MM_DTYPE", "bf16")
# fp32r requires an even innermost moving count -> keep the dead v=33 col;
# bf16/f32 have no such restriction.
NV = 34 if MM_DTYPE == "f32r" else 33
NT = 11 * NV          # N per matmul: 11 u-rows x NV v-cols


def _split_multi_sync(nc):
    """The walrus in this env allows only ONE sync-wait per instruction.
    Hoist extra waits onto same-engine InstNoOp carriers placed just before
    the owning instruction (sequential waits on one engine == AND)."""
    ctr = 0
    for f in nc.m.functions:
        for bb in f.blocks:
            insts = list(bb.instructions)
            out = []
            changed = False
            for inst in insts:
                si = inst.sync_info
                waits = list(si.on_wait) if si and si.on_wait else []
                if len(waits) > 1:
                    for w in waits[:-1]:
                        nop = mybir.InstNoOp(name=f"waitnop-{ctr}", ins=[], outs=[])
                        ctr += 1
                        nop.engine = inst.engine
                        nop.sync_info = mybir.SyncInfo(on_wait=[w], on_update=[])
                        out.append(nop)
                    si.on_wait = [waits[-1]]
                    changed = True
                out.append(inst)
            if changed:
                bb.instructions = out
    return ctr


def _build_nc():
    f32 = mybir.dt.float32
    # float32r has identical bits/np-dtype to float32; the PE runs its
    # matmuls at 1 cyc/row (vs 4 for exact f32). The BIR verifier requires
    # every producer of an f32r matmul operand to *output* f32r, so the
    # W/X dram params and sbuf tiles are typed f32r end-to-end.
    mmdt = {"f32r": mybir.dt.float32r, "bf16": mybir.dt.bfloat16}.get(MM_DTYPE, f32)
    nc = bass.Bass(enable_partition_id=False)
    # W chunks pre-gathered host-side, partition-major: [p, ci, (c,ry,rx)].
    # 16KB contiguous per partition per 4-chunk quad -> large DMA packets
    # (4KB runs cost ~400ns/packet overhead and cap DMA at ~160 GB/s).
    w4 = nc.declare_dram_parameter("w4", [128, 32, C * 64], mmdt, isOutput=False)
    mT = nc.declare_dram_parameter("mT", [128, 32, 64], f32, isOutput=False)
    # X is y-major [p, yy, a, xx] and loads in 3 phase-aligned y-slabs:
    # phase n only reads rows [11n, 11n+13), so the first matmuls need just
    # slab 0 (rows 0..13) instead of the whole 5MB tensor.
    xp = nc.declare_dram_parameter("xp", [128, 34, 8, 36], mmdt, isOutput=False)
    # a-major copy of slab-0 rows so the startup slivers are contiguous
    xp0 = nc.declare_dram_parameter("xp0", [128, 8, 13, 36], mmdt, isOutput=False)
    # out: phase-major [n, p, m, NT] so each phase writes one 12KB/partition DMA
    outT = nc.declare_dram_parameter("outT", [3, 128, 8, NT], f32, isOutput=True)

    with tile.TileContext(nc) as tc:
        with (
            tc.tile_pool(name="xpp", bufs=1) as xpp,
            tc.tile_pool(name="wp", bufs=8) as wp,
            tc.tile_pool(name="mp", bufs=1) as mp,
            tc.tile_pool(name="onp", bufs=4) as onp,
            tc.tile_pool(name="op", bufs=2) as op,
            tc.tile_pool(name="pp", bufs=8, space="PSUM") as pp,
        ):
            # Startup-critical DMAs lead each queue's FIFO: the first matmul
            # needs only s0a (a=0; chunks 0-3 all read a=0) + mta0 + chunk 0
            # of W on the sync queue.
            # one tile per y-slab (separate tiles: dependency tracking is
            # whole-tile, and deferred slabs must not deadlock phase-0 reads).
            # Slabs overlap by 2 rows so each phase reads within one slab.
            SLABS = ((0, 13), (11, 24), (22, 34))
            s0a = xpp.tile([128, 1, 13, 36], mmdt)
            nc.scalar.dma_start(s0a[:], xp0[:, 0:1])
            mta0 = mp.tile([128, 4, 64], f32)
            nc.scalar.dma_start(mta0[:], mT[:, 0:4])
            s0b = xpp.tile([128, 7, 13, 36], mmdt)
            nc.scalar.dma_start(s0b[:], xp0[:, 1:8])
            mta1 = mp.tile([128, 28, 64], f32)
            nc.scalar.dma_start(mta1[:], mT[:, 4:32])

            def mask_ap(ci):
                return mta0[:, ci, :] if ci < 4 else mta1[:, ci - 4, :]
            slab_dmas = [None]
            slabs = [None]
            for si, (y0, y1) in list(enumerate(SLABS))[1:]:
                st = xpp.tile([128, y1 - y0, 8, 36], mmdt, name=f"slab_{si}")
                slab_dmas.append(nc.scalar.dma_start(st[:], xp[:, y0:y1]))
                slabs.append(st)

            def rhs_ap(n, a, sy, sx):
                y0 = 11 * n + 1 - sy - SLABS[n][0]
                x0 = 1 - sx
                if n == 0:
                    t, aa = (s0a, a) if a < 1 else (s0b, a - 1)
                    return t[:, aa, y0:y0 + 11, x0:x0 + NV]
                return slabs[n][:, y0:y0 + 11, a, x0:x0 + NV]

            # W streams in 8 quads of 4 chunks; mask-mul fused per chunk.
            # Chunk 0 gets its OWN tile: dependency tracking is whole-tile,
            # so as a quad slice its first matmul would wait on all 4 muls.
            wc0 = wp.tile([128, 1, C * 64], mmdt, name="wc0")
            nc.sync.dma_start(wc0[:], w4[:, 0:1, :])
            quads = []
            quad_dmas = []
            for g in range(8):
                wq = wp.tile([128, 4, C * 64], mmdt, tag="w", name=f"wq_{g}")
                if g == 0:
                    dma = nc.sync.dma_start(wq[:, 1:4, :], w4[:, 1:4, :])
                else:
                    dma = nc.sync.dma_start(wq[:], w4[:, 4 * g:4 * g + 4, :])
                quad_dmas.append(dma)
                for j in range(4):
                    ci = 4 * g + j
                    om = onp.tile([128, 64], f32, tag="om")
                    nc.vector.tensor_scalar(
                        out=om[:], in0=mask_ap(ci), scalar1=-1.0, scalar2=1.0,
                        op0=mybir.AluOpType.mult, op1=mybir.AluOpType.add,
                    )
                    wt_dst = wc0 if ci == 0 else wq
                    jj = 0 if ci == 0 else j
                    wv = wt_dst[:, jj, :].rearrange("p (c f) -> p c f", c=C)
                    in0 = wv.bitcast(f32) if MM_DTYPE == "f32r" else wv
                    nc.vector.tensor_tensor(
                        out=wv, in0=in0,
                        in1=om[:, None, :].broadcast_to([128, C, 64]),
                        op=mybir.AluOpType.mult,
                    )
                quads.append(wq)

            def lhsT(ci, m):
                if ci == 0:
                    return wc0[:, 0, 128 * m:128 * (m + 1)]
                return quads[ci // 4][:, ci % 4, 128 * m:128 * (m + 1)]

            # Phase 0 (n=0) is chunk-outer with 8 live psum groups so the PE
            # consumes W quads as they stream (no all-32-chunks stall).
            # Phases 1-2 run m-outer (all data resident) so each group's
            # psum copy + output DMA overlaps the next group's matmuls.
            for n in range(3):
                osb = op.tile([128, 8, NT], f32, tag="o", name=f"osb_{n}")
                if n == 0:
                    pss = [pp.tile([128, NT], f32, tag="ps", name=f"ps_{n}_{i}")
                           for i in range(8)]
                    mm0 = {}
                    for ci, (a, sy, sx) in enumerate(CHUNKS):
                        rhs = rhs_ap(n, a, sy, sx)
                        for m in range(8):
                            mm0[ci, m] = nc.tensor.matmul(
                                pss[m][:], lhsT(ci, m), rhs,
                                start=(ci == 0), stop=(ci == 31),
                            )
                    # Defer the late W quads / X slabs behind PE progress so
                    # the startup-critical DMAs (mta, quad0, slab0) get the
                    # full DMA bandwidth during the ramp.
                    from concourse.tile_rust import add_dep_helper
                    add_dep_helper(quad_dmas[2].ins, mm0[0, 0].ins,
                                   sync=True, reason="stream quads behind PE")
                    for g in range(3, 8):
                        add_dep_helper(quad_dmas[g].ins, mm0[4 * (g - 3) + 3, 7].ins,
                                       sync=True, reason="stream quads behind PE")
                    add_dep_helper(slab_dmas[1].ins, mm0[3, 7].ins,
                                   sync=True, reason="slab1 after early phase0")
                    add_dep_helper(slab_dmas[2].ins, mm0[11, 7].ins,
                                   sync=True, reason="slab2 after mid phase0")
                    for m in range(8):
                        nc.vector.tensor_copy(osb[:, m, :], pss[m][:])
                else:
                    for m in range(8):
                        ps = pp.tile([128, NT], f32, tag="ps", name=f"ps_{n}_{m}")
                        for ci, (a, sy, sx) in enumerate(CHUNKS):
                            rhs = rhs_ap(n, a, sy, sx)
                            nc.tensor.matmul(
                                ps[:], lhsT(ci, m), rhs,
                                start=(ci == 0), stop=(ci == 31),
                            )
                        nc.vector.tensor_copy(osb[:, m, :], ps[:])
                # split the phase writeback so earlier pieces overlap the
                # remaining matmuls (finest on the last phase to shrink tail)
                if n < 2:
                    nc.scalar.dma_start(outT[n, :, 0:4], osb[:, 0:4])
                    nc.scalar.dma_start(outT[n, :, 4:8], osb[:, 4:8])
                else:
                    for mm2 in range(0, 8, 2):
                        nc.scalar.dma_start(outT[n, :, mm2:mm2 + 2],
                                            osb[:, mm2:mm2 + 2])

    _split_multi_sync(nc)
    return nc


def _host_prep(b_ch, mask_b, cos_b):
    """b_ch (16,256,256) f32, mask_b (256,256) f32, cos_b (1024,32,32) f32
    -> dict of device inputs (layout/gather glue only)."""
    bpad = np.pad(b_ch, ((0, 0), (PD, PD), (PD, PD)), mode="edge")
    mpad = np.pad(mask_b, ((PD, PD), (PD, PD)), mode="edge")
    # block layout [bi*33+bj, (c,ry,rx)]
    bT = bpad.reshape(C, 33, 8, 33, 8).transpose(1, 3, 0, 2, 4).reshape(33 * 33, C * 64)
    mTb = mpad.reshape(33, 8, 33, 8).transpose(0, 2, 1, 3).reshape(33 * 33, 64)
    # unfold-as-shifted-block-views: chunk (a,sy,sx), partition p=32*pi+pj
    # reads block row (4a+pi+sy)*33 + (pj+sx).  Pre-gather partition-major.
    pi, pj = np.arange(4)[:, None], np.arange(32)[None, :]
    rows = np.stack([((4 * a + pi + sy) * 33 + (pj + sx)).reshape(128)
                     for (a, sy, sx) in CHUNKS], axis=1)        # [128, 32]
    w4 = np.ascontiguousarray(bT[rows])                          # [128,32,1024]
    mT = np.ascontiguousarray(mTb[rows])                         # [128,32,64]
    xp = np.zeros((1024, 34, 36), np.float32)
    xp[:, 1:33, 1:33] = cos_b
    # [l=128a+p, yy, xx] -> [p, yy, a, xx]; plus an a-major slab-0 copy
    xpb = xp.reshape(8, 128, 34, 36)
    xp0 = np.ascontiguousarray(xpb[:, :, 0:13, :].transpose(1, 0, 2, 3))
    xp = np.ascontiguousarray(xpb.transpose(1, 2, 0, 3))
    if MM_DTYPE == "bf16":
        import ml_dtypes
        w4 = w4.astype(ml_dtypes.bfloat16)
        xp = xp.astype(ml_dtypes.bfloat16)
        xp0 = xp0.astype(ml_dtypes.bfloat16)
    return {"w4": w4, "mT": mT, "xp": xp, "xp0": xp0}


def _unshard(outT):
    # outT [3, 128, 8, 11*NV] -> [(c,ry,rx)=128m+p, u=11n+u', v] -> (16,256,256)
    t = outT.reshape(3, 128, 8, 11, NV).transpose(2, 1, 0, 3, 4).reshape(1024, 33, NV)
    t = t[:, :, :33].reshape(C, 8, 8, 33, 33).transpose(0, 3, 1, 4, 2)
    return t.reshape(C, 264, 264)[:, 4:260, 4:260]


_RUN_KW = {}   # test harness may inject e.g. trace=True
_LAST_RESULTS = [None]
_NC_CACHE = {}


def _get_nc():
    nc = _NC_CACHE.get(MM_DTYPE)
    if nc is None:
        nc = _NC_CACHE[MM_DTYPE] = _build_nc()
    return nc


def kernel(cos_similar, b, mask):
    cos_similar = np.ascontiguousarray(np.asarray(cos_similar, dtype=np.float32))
    b = np.ascontiguousarray(np.asarray(b, dtype=np.float32))
    mask = np.ascontiguousarray(np.asarray(mask, dtype=np.float32))

    in_maps = []
    for core in range(N_CORES):
        batch, half = core // 2, core % 2
        ch0 = C * half
        in_maps.append(_host_prep(
            b[batch, ch0:ch0 + C], mask[batch, 0], cos_similar[batch]))

    nc = _get_nc()
    res = run_bass_kernel_spmd(nc, in_maps, list(range(N_CORES)), **_RUN_KW)
    _LAST_RESULTS[0] = res

    out = np.empty((4, 32, 256, 256), np.float32)
    for core in range(N_CORES):
        batch, half = core // 2, core % 2
        ch0 = C * half
        out[batch, ch0:ch0 + C] = _unshard(res.results[core]["outT"])
    return out
